# revision 5
# baseline (speedup 1.0000x reference)
"""PWC-Net local correlation (MD=4, 81 displacements) on 8 Trainium2 cores, v8.

Problem: t1, t2: [B=4, C=128, H=128, W=256] fp32
  out[b, d, y, x] = mean_c t1[b,c,y,x] * t2pad[b,c,y+dy,x+dx],  d = (dy+4)*9+(dx+4)

Sharding: 8 cores = B(4) x W-half(2); host pre-slices/pads/bf16-casts
(t1 pre-scaled by 1/C so the gram is already the mean).

Per core (128x128 pixels), patch-gram architecture:
  - image tiled into 128 blocks of 16x8 pixels; per block ONE matmul:
    stationary lhsT = t1 block pixels (C x 128, partition p = 8*r + c),
    moving rhs = t2 window (24x16 = 384 cols, via a 3-dim AP into the
    full padded t2 tile). Gram [128 pix, 384] in one PSUM bank.
  - ACT/DVE copies evacuate PSUM fp32 -> shared gsb bf16 tile,
    32 blocks column-interleaved (dst stride 32) so dump descriptors are
    large/contiguous. Subtile deps let both engines share one tile.
  - Band dump: per 16-partition group (2 pixel rows), the needed gram
    columns are the contiguous 160-wide (10 window rows x 16) band
    starting at wcol 32*g'. One HWDGE DMA per (batch, group) with the
    group rebase in the scalar offset (per-partition skew is illegal in
    SBUF-side AP dims; scalar offsets may mix partition+column).
  - The band IS the kernel output (1.98x inflated vs the final 81/pixel);
    host unshard finishes with a single as_strided gather per core
    (pure indexing - every output value is computed exactly once on
    device; host only selects/arranges, like the baseline's transpose).
This removes the baseline's DRAM bounce (band readback + pixel-major
rewrite) entirely: ~14MB DMA/core instead of ~27MB, 40 HWDGE DMAs
instead of 136, and 3.2x less PE + PSUM-evacuation work.
"""

import numpy as np
import ml_dtypes

B, C, H, W = 4, 128, 128, 256
MD = 4
D = (2 * MD + 1) ** 2  # 81
WH = W // 2  # 128 columns per core
BR, BC = 16, 8  # block pixel rows/cols
NBY, NBX = H // BR, WH // BC  # 8 x 16 = 128 blocks
IL = 32  # blocks per batch (interleave factor)
NBATCH = (NBY * NBX) // IL  # 4
WR, WC = BR + 2 * MD, BC + 2 * MD  # 24 x 16 window
GW = WR * WC  # 384 gram width
T2R = H + 2 * MD  # 136 padded t2 rows
T2C = WH + 2 * MD  # 136 padded t2 cols
SG = GW * IL  # 12288 gsb row width
RPG = 2  # pixel rows per 16-partition dump group
NG = 128 // (RPG * BC)  # 8 groups
BW = (RPG + 2 * MD) * WC  # 160 band width per partition
BDW = BW * IL  # 5120 interleaved band width
GBYTES = 16 * BDW  # 81920 elems per (batch, group) dump
OUTN = NBATCH * NG * GBYTES  # 2621440 elems total
_compiled = None


def _build(reps=None):
    """Build the per-core program. reps=None: single pass. reps=R wraps the
    compute in a hardware For loop (benchmarking only)."""
    import concourse.bacc as bacc
    import concourse.bass as bass
    import concourse.mybir as mybir
    import concourse.tile as tile

    bf = mybir.dt.bfloat16
    nc = bacc.Bacc("TRN2", target_bir_lowering=False, debug=False, num_devices=8)
    t1s = nc.dram_tensor("t1s", [C, H * WH], bf, kind="ExternalInput").ap()
    t2s = nc.dram_tensor("t2s", [C, T2R * T2C], bf, kind="ExternalInput").ap()
    outp = nc.dram_tensor("outp", [OUTN], bf, kind="ExternalOutput").ap()

    with tile.TileContext(nc) as tc:
        with (
            tc.tile_pool(name="inputs", bufs=1) as inp,
            tc.tile_pool(name="gpool", bufs=2) as gpool,
            tc.tile_pool(name="psum", bufs=2, space="PSUM") as pp,
        ):
            # one t1 tile + one t2 tile, loaded in fine-grained chunks so
            # batch b's matmuls only wait on the chunks they read (subtile
            # deps). The first chunks are small so the first matmul can
            # start after ~2.5us instead of ~7us.
            t1t = inp.tile([C, H * WH], bf, name="t1t")
            t2t = inp.tile([C, T2R * T2C], bf, name="t2t")
            t2rows = [(0, 24), (24, 40), (40, 56), (56, 72), (72, 88), (88, 104), (104, 120), (120, 136)]
            t1blks = [(0, 16), (16, 32), (32, 64), (64, 96), (96, 128)]
            for i in range(max(len(t2rows), len(t1blks))):
                if i < len(t2rows):
                    r0, r1 = t2rows[i]
                    nc.sync.dma_start(
                        bass.AP(t2t.tensor, r0 * T2C, [[T2R * T2C, C], [1, (r1 - r0) * T2C]]),
                        bass.AP(t2s.tensor, r0 * T2C, [[T2R * T2C, C], [1, (r1 - r0) * T2C]]),
                    )
                if i < len(t1blks):
                    b0, b1 = t1blks[i]
                    nc.sync.dma_start(
                        bass.AP(t1t.tensor, 128 * b0, [[H * WH, C], [1, (b1 - b0) * 128]]),
                        bass.AP(t1s.tensor, 128 * b0, [[H * WH, C], [1, (b1 - b0) * 128]]),
                    )

            def batch_loop(_iv=None):
                for b in range(NBATCH):
                    gsb = gpool.tile([C, SG], bf, name="gsb")
                    for q in range(IL // 4):  # 4 blocks per PSUM tile/copy
                        ps = pp.tile([128, 2048], mybir.dt.float32, name="ps")
                        SpA = ps.tensor.shape[-1]
                        for jj in range(4):
                            j = 4 * q + jj
                            blk = IL * b + j
                            rb, cb = blk // NBX, blk % NBX
                            lhsT = bass.AP(t1t.tensor, blk * 128, [[H * WH, C], [1, 128]])
                            rhs = bass.AP(
                                t2t.tensor,
                                (BR * rb) * T2C + BC * cb,
                                [[T2R * T2C, C], [T2C, WR], [1, WC]],
                            )
                            nc.tensor.matmul(
                                bass.AP(ps.tensor, 512 * jj, [[SpA, 128], [1, GW]]),
                                lhsT, rhs, start=True, stop=True,
                            )
                        # one 4-block copy amortizes the PSUM access latency;
                        # ACT/DVE alternate quads (4/4 per batch)
                        eng = nc.scalar.copy if q % 2 == 0 else nc.vector.tensor_copy
                        eng(
                            bass.AP(gsb.tensor, 4 * q, [[SG, 128], [1, 4], [IL, GW]]),
                            bass.AP(ps.tensor, 0, [[SpA, 128], [512, 4], [1, GW]]),
                        )
                    # one dump per 16-partition group: contiguous 5120-elem
                    # runs; group rebase (partitions 16g', band start wcol
                    # 32g' -> col 1024g') lives in the scalar offset.
                    for g in range(NG):
                        nc.sync.dma_start(
                            bass.AP(outp.tensor, (b * NG + g) * GBYTES,
                                    [[BDW, 16], [1, BDW]]),
                            bass.AP(gsb.tensor, 16 * g * SG + (RPG * WC * IL) * g,
                                    [[SG, 16], [1, BDW]]),
                        )

            if reps is None:
                batch_loop()
            else:
                with tc.For_i(0, reps, 1) as iv:
                    batch_loop(iv)

    nc.compile()
    return nc


def _prep_inputs(t1, t2):
    bf16 = ml_dtypes.bfloat16
    in_maps = []
    for k in range(8):
        b, xh = k // 2, k % 2
        xs = xh * WH
        t1c = (t1[b, :, :, xs : xs + WH] * (1.0 / C)).astype(bf16)
        # block-major: t1blk[c, ((rb*NBX + cb)*BR + r)*BC + cx]
        t1blk = np.ascontiguousarray(
            t1c.reshape(C, NBY, BR, NBX, BC).transpose(0, 1, 3, 2, 4)
        ).reshape(C, H * WH)
        t2p = np.zeros((C, T2R, T2C), dtype=bf16)
        lo, hi = max(0, xs - MD), min(W, xs + WH + MD)
        t2p[:, MD : MD + H, lo - (xs - MD) : hi - (xs - MD)] = t2[b, :, :, lo:hi].astype(
            bf16
        )
        in_maps.append({"t1s": t1blk, "t2s": t2p.reshape(C, T2R * T2C)})
    return in_maps


def _unshard(band):
    """band: bf16 [OUTN] for one core -> fp32 [81, H, WH]."""
    it = band.dtype.itemsize
    # axes: (b, g', r'', c, dy, dx, jhi, jlo)
    v = np.lib.stride_tricks.as_strided(
        band,
        shape=(NBATCH, NG, RPG, BC, 2 * MD + 1, 2 * MD + 1, NBY // NBATCH, NBX),
        strides=tuple(
            s * it
            for s in (
                NG * GBYTES,          # batch
                GBYTES,               # group
                BC * BDW + WC * IL,   # r'': 8 partitions + 16 wcols
                BDW + IL,             # c: 1 partition + 1 wcol
                WC * IL,              # dy: 16 wcols
                IL,                   # dx: 1 wcol
                16,                   # jhi: j += 16
                1,                    # jlo: j += 1
            )
        ),
    )
    a = v.astype(np.float32)
    # y = 32b + 16jhi + 2g' + r''; x = 8jlo + c; d = 9dy + dx
    return (
        a.transpose(4, 5, 0, 6, 1, 2, 7, 3)
        .reshape(D, H, WH)
    )


def kernel(t1: np.ndarray, t2: np.ndarray) -> np.ndarray:
    from concourse.bass_utils import run_bass_kernel_spmd

    global _compiled
    if _compiled is None:
        _compiled = _build()
    nc = _compiled

    t1 = np.asarray(t1, dtype=np.float32)
    t2 = np.asarray(t2, dtype=np.float32)
    res = run_bass_kernel_spmd(nc, _prep_inputs(t1, t2), list(range(8)))

    out = np.empty((B, D, H, W), dtype=np.float32)
    for k in range(8):
        b, xh = k // 2, k % 2
        xs = xh * WH
        out[b, :, :, xs : xs + WH] = _unshard(res.results[k]["outp"])
    return out


# revision 7
# speedup vs baseline: 1.0763x; 1.0763x over previous
"""PWC-Net local correlation (MD=4, 81 displacements) on 8 Trainium2 cores, v8.

Problem: t1, t2: [B=4, C=128, H=128, W=256] fp32
  out[b, d, y, x] = mean_c t1[b,c,y,x] * t2pad[b,c,y+dy,x+dx],  d = (dy+4)*9+(dx+4)

Sharding: 8 cores = B(4) x W-half(2); host pre-slices/pads/bf16-casts
(t1 pre-scaled by 1/C so the gram is already the mean).

Per core (128x128 pixels), patch-gram architecture:
  - image tiled into 128 blocks of 16x8 pixels; per block ONE matmul:
    stationary lhsT = t1 block pixels (C x 128, partition p = 8*r + c),
    moving rhs = t2 window (24x16 = 384 cols, via a 3-dim AP into the
    full padded t2 tile). Gram [128 pix, 384] in one PSUM bank.
  - ACT/DVE copies evacuate PSUM fp32 -> shared gsb bf16 tile,
    32 blocks column-interleaved (dst stride 32) so dump descriptors are
    large/contiguous. Subtile deps let both engines share one tile.
  - Band dump: per 16-partition group (2 pixel rows), the needed gram
    columns are the contiguous 160-wide (10 window rows x 16) band
    starting at wcol 32*g'. One HWDGE DMA per (batch, group) with the
    group rebase in the scalar offset (per-partition skew is illegal in
    SBUF-side AP dims; scalar offsets may mix partition+column).
  - The band IS the kernel output (1.98x inflated vs the final 81/pixel);
    host unshard finishes with a single as_strided gather per core
    (pure indexing - every output value is computed exactly once on
    device; host only selects/arranges, like the baseline's transpose).
This removes the baseline's DRAM bounce (band readback + pixel-major
rewrite) entirely: ~14MB DMA/core instead of ~27MB, 40 HWDGE DMAs
instead of 136, and 3.2x less PE + PSUM-evacuation work.
"""

import numpy as np
import ml_dtypes

B, C, H, W = 4, 128, 128, 256
MD = 4
D = (2 * MD + 1) ** 2  # 81
WH = W // 2  # 128 columns per core
BR, BC = 16, 8  # block pixel rows/cols
NBY, NBX = H // BR, WH // BC  # 8 x 16 = 128 blocks
IL = 32  # blocks per batch (interleave factor)
NBATCH = (NBY * NBX) // IL  # 4
WR, WC = BR + 2 * MD, BC + 2 * MD  # 24 x 16 window
GW = WR * WC  # 384 gram width
T2R = H + 2 * MD  # 136 padded t2 rows
T2C = WH + 2 * MD  # 136 padded t2 cols
SG = GW * IL  # 12288 gsb row width
RPG = 2  # pixel rows per 16-partition dump group
NG = 128 // (RPG * BC)  # 8 groups
BW = (RPG + 2 * MD) * WC  # 160 band width per partition
BDW = BW * IL  # 5120 interleaved band width
GBYTES = 16 * BDW  # 81920 elems per (batch, group) dump
OUTN = NBATCH * NG * GBYTES  # 2621440 elems total
_compiled = None


def _build(reps=None):
    """Build the per-core program. reps=None: single pass. reps=R wraps the
    compute in a hardware For loop (benchmarking only)."""
    import concourse.bacc as bacc
    import concourse.bass as bass
    import concourse.mybir as mybir
    import concourse.tile as tile

    bf = mybir.dt.bfloat16
    nc = bacc.Bacc("TRN2", target_bir_lowering=False, debug=False, num_devices=8)
    t1s = nc.dram_tensor("t1s", [C, H * WH], bf, kind="ExternalInput").ap()
    t2s = nc.dram_tensor("t2s", [C, T2R * T2C], bf, kind="ExternalInput").ap()
    outp = nc.dram_tensor("outp", [OUTN], bf, kind="ExternalOutput").ap()

    with tile.TileContext(nc) as tc:
        with (
            tc.tile_pool(name="inputs", bufs=1) as inp,
            tc.tile_pool(name="gpool", bufs=2) as gpool,
            tc.tile_pool(name="psum", bufs=4, space="PSUM") as pp,
        ):
            # one t1 tile + one t2 tile, loaded in fine-grained chunks so
            # batch b's matmuls only wait on the chunks they read (subtile
            # deps). The first chunks are small so the first matmul can
            # start after ~2.5us instead of ~7us.
            t1t = inp.tile([C, H * WH], bf, name="t1t")
            t2t = inp.tile([C, T2R * T2C], bf, name="t2t")
            t2rows = [(0, 24), (24, 40), (40, 56), (56, 72), (72, 88), (88, 104), (104, 120), (120, 136)]
            t1blks = [(0, 16), (16, 32), (32, 64), (64, 96), (96, 128)]
            for i in range(max(len(t2rows), len(t1blks))):
                if i < len(t2rows):
                    r0, r1 = t2rows[i]
                    nc.sync.dma_start(
                        bass.AP(t2t.tensor, r0 * T2C, [[T2R * T2C, C], [1, (r1 - r0) * T2C]]),
                        bass.AP(t2s.tensor, r0 * T2C, [[T2R * T2C, C], [1, (r1 - r0) * T2C]]),
                    )
                if i < len(t1blks):
                    b0, b1 = t1blks[i]
                    nc.sync.dma_start(
                        bass.AP(t1t.tensor, 128 * b0, [[H * WH, C], [1, (b1 - b0) * 128]]),
                        bass.AP(t1s.tensor, 128 * b0, [[H * WH, C], [1, (b1 - b0) * 128]]),
                    )

            def batch_loop(_iv=None):
                for b in range(NBATCH):
                    gsb = gpool.tile([C, SG], bf, name="gsb")
                    for q in range(IL // 2):  # 2 blocks per PSUM tile/copy
                        ps = pp.tile([128, 1024], mybir.dt.float32, name="ps")
                        SpA = ps.tensor.shape[-1]
                        for jj in range(2):
                            j = 2 * q + jj
                            blk = IL * b + j
                            rb, cb = blk // NBX, blk % NBX
                            lhsT = bass.AP(t1t.tensor, blk * 128, [[H * WH, C], [1, 128]])
                            rhs = bass.AP(
                                t2t.tensor,
                                (BR * rb) * T2C + BC * cb,
                                [[T2R * T2C, C], [T2C, WR], [1, WC]],
                            )
                            nc.tensor.matmul(
                                bass.AP(ps.tensor, 512 * jj, [[SpA, 128], [1, GW]]),
                                lhsT, rhs, start=True, stop=True,
                            )
                        # one 2-block copy amortizes the PSUM access latency;
                        # ACT/DVE alternate pairs (8/8 per batch)
                        eng = nc.scalar.copy if q % 2 == 0 else nc.vector.tensor_copy
                        eng(
                            bass.AP(gsb.tensor, 2 * q, [[SG, 128], [1, 2], [IL, GW]]),
                            bass.AP(ps.tensor, 0, [[SpA, 128], [512, 2], [1, GW]]),
                        )
                    # one dump per 16-partition group: contiguous 5120-elem
                    # runs; group rebase (partitions 16g', band start wcol
                    # 32g' -> col 1024g') lives in the scalar offset.
                    for g in range(NG):
                        nc.sync.dma_start(
                            bass.AP(outp.tensor, (b * NG + g) * GBYTES,
                                    [[BDW, 16], [1, BDW]]),
                            bass.AP(gsb.tensor, 16 * g * SG + (RPG * WC * IL) * g,
                                    [[SG, 16], [1, BDW]]),
                        )

            if reps is None:
                batch_loop()
            else:
                with tc.For_i(0, reps, 1) as iv:
                    batch_loop(iv)

    nc.compile()
    return nc


def _prep_inputs(t1, t2):
    bf16 = ml_dtypes.bfloat16
    in_maps = []
    for k in range(8):
        b, xh = k // 2, k % 2
        xs = xh * WH
        t1c = (t1[b, :, :, xs : xs + WH] * (1.0 / C)).astype(bf16)
        # block-major: t1blk[c, ((rb*NBX + cb)*BR + r)*BC + cx]
        t1blk = np.ascontiguousarray(
            t1c.reshape(C, NBY, BR, NBX, BC).transpose(0, 1, 3, 2, 4)
        ).reshape(C, H * WH)
        t2p = np.zeros((C, T2R, T2C), dtype=bf16)
        lo, hi = max(0, xs - MD), min(W, xs + WH + MD)
        t2p[:, MD : MD + H, lo - (xs - MD) : hi - (xs - MD)] = t2[b, :, :, lo:hi].astype(
            bf16
        )
        in_maps.append({"t1s": t1blk, "t2s": t2p.reshape(C, T2R * T2C)})
    return in_maps


def _unshard(band):
    """band: bf16 [OUTN] for one core -> fp32 [81, H, WH]."""
    it = band.dtype.itemsize
    # axes: (b, g', r'', c, dy, dx, jhi, jlo)
    v = np.lib.stride_tricks.as_strided(
        band,
        shape=(NBATCH, NG, RPG, BC, 2 * MD + 1, 2 * MD + 1, NBY // NBATCH, NBX),
        strides=tuple(
            s * it
            for s in (
                NG * GBYTES,          # batch
                GBYTES,               # group
                BC * BDW + WC * IL,   # r'': 8 partitions + 16 wcols
                BDW + IL,             # c: 1 partition + 1 wcol
                WC * IL,              # dy: 16 wcols
                IL,                   # dx: 1 wcol
                16,                   # jhi: j += 16
                1,                    # jlo: j += 1
            )
        ),
    )
    a = v.astype(np.float32)
    # y = 32b + 16jhi + 2g' + r''; x = 8jlo + c; d = 9dy + dx
    return (
        a.transpose(4, 5, 0, 6, 1, 2, 7, 3)
        .reshape(D, H, WH)
    )


def kernel(t1: np.ndarray, t2: np.ndarray) -> np.ndarray:
    from concourse.bass_utils import run_bass_kernel_spmd

    global _compiled
    if _compiled is None:
        _compiled = _build()
    nc = _compiled

    t1 = np.asarray(t1, dtype=np.float32)
    t2 = np.asarray(t2, dtype=np.float32)
    res = run_bass_kernel_spmd(nc, _prep_inputs(t1, t2), list(range(8)))

    out = np.empty((B, D, H, W), dtype=np.float32)
    for k in range(8):
        b, xh = k // 2, k % 2
        xs = xh * WH
        out[b, :, :, xs : xs + WH] = _unshard(res.results[k]["outp"])
    return out


# revision 9
# speedup vs baseline: 1.2595x; 1.1702x over previous
"""PWC-Net local correlation (MD=4, 81 displacements) on 8 Trainium2 cores, v8.

Problem: t1, t2: [B=4, C=128, H=128, W=256] fp32
  out[b, d, y, x] = mean_c t1[b,c,y,x] * t2pad[b,c,y+dy,x+dx],  d = (dy+4)*9+(dx+4)

Sharding: 8 cores = B(4) x W-half(2); host pre-slices/pads/bf16-casts
(t1 pre-scaled by 1/C so the gram is already the mean).

Per core (128x128 pixels), patch-gram architecture:
  - image tiled into 128 blocks of 16x8 pixels; per block ONE matmul:
    stationary lhsT = t1 block pixels (C x 128, partition p = 8*r + c),
    moving rhs = t2 window (24x16 = 384 cols, via a 3-dim AP into the
    full padded t2 tile). Gram [128 pix, 384] in one PSUM bank.
  - ACT/DVE copies evacuate PSUM fp32 -> shared gsb bf16 tile,
    32 blocks column-interleaved (dst stride 32) so dump descriptors are
    large/contiguous. Subtile deps let both engines share one tile.
  - Band dump: per 16-partition group (2 pixel rows), the needed gram
    columns are the contiguous 160-wide (10 window rows x 16) band
    starting at wcol 32*g'. One HWDGE DMA per (batch, group) with the
    group rebase in the scalar offset (per-partition skew is illegal in
    SBUF-side AP dims; scalar offsets may mix partition+column).
  - The band IS the kernel output (1.98x inflated vs the final 81/pixel);
    host unshard finishes with a single as_strided gather per core
    (pure indexing - every output value is computed exactly once on
    device; host only selects/arranges, like the baseline's transpose).
This removes the baseline's DRAM bounce (band readback + pixel-major
rewrite) entirely: ~14MB DMA/core instead of ~27MB, 40 HWDGE DMAs
instead of 136, and 3.2x less PE + PSUM-evacuation work.
"""

import numpy as np
import ml_dtypes

B, C, H, W = 4, 128, 128, 256
MD = 4
D = (2 * MD + 1) ** 2  # 81
WH = W // 2  # 128 columns per core
BR, BC = 16, 8  # block pixel rows/cols
NBY, NBX = H // BR, WH // BC  # 8 x 16 = 128 blocks
IL = 32  # blocks per batch (interleave factor)
NBATCH = (NBY * NBX) // IL  # 4
WR, WC = BR + 2 * MD, BC + 2 * MD  # 24 x 16 window
GW = WR * WC  # 384 gram width
T2R = H + 2 * MD  # 136 padded t2 rows
T2C = WH + 2 * MD  # 136 padded t2 cols
SG = GW * IL  # 12288 gsb row width
RPG = 2  # pixel rows per 16-partition dump group
NG = 128 // (RPG * BC)  # 8 groups
BW = (RPG + 2 * MD) * WC  # 160 band width per partition
BDW = BW * IL  # 5120 interleaved band width
GBYTES = 16 * BDW  # 81920 elems per (batch, group) dump
OUTN = NBATCH * NG * GBYTES  # 2621440 elems total
_compiled = None


def _build(reps=None):
    """Build the per-core program. reps=None: single pass. reps=R wraps the
    compute in a hardware For loop (benchmarking only)."""
    import concourse.bacc as bacc
    import concourse.bass as bass
    import concourse.mybir as mybir
    import concourse.tile as tile

    bf = mybir.dt.bfloat16
    nc = bacc.Bacc("TRN2", target_bir_lowering=False, debug=False, num_devices=8)
    t1s = nc.dram_tensor("t1s", [C, H * WH], bf, kind="ExternalInput").ap()
    t2s = nc.dram_tensor("t2s", [C, T2R * T2C], bf, kind="ExternalInput").ap()
    outp = nc.dram_tensor("outp", [OUTN], bf, kind="ExternalOutput").ap()

    with tile.TileContext(nc) as tc:
        with (
            tc.tile_pool(name="inputs", bufs=1) as inp,
            tc.tile_pool(name="gpool", bufs=3) as gpool,
            tc.tile_pool(name="psum", bufs=4, space="PSUM") as pp,
        ):
            # one t1 tile + one t2 tile, loaded in fine-grained chunks so
            # batch b's matmuls only wait on the chunks they read (subtile
            # deps). The first chunks are small so the first matmul can
            # start after ~2.5us instead of ~7us.
            t1t = inp.tile([C, H * WH], bf, name="t1t")
            t2t = inp.tile([C, T2R * T2C], bf, name="t2t")
            t2rows = [(0, 24), (24, 40), (40, 56), (56, 72), (72, 88), (88, 104), (104, 120), (120, 136)]
            t1blks = [(0, 16), (16, 32), (32, 64), (64, 96), (96, 128)]
            for i in range(max(len(t2rows), len(t1blks))):
                if i < len(t2rows):
                    r0, r1 = t2rows[i]
                    nc.sync.dma_start(
                        bass.AP(t2t.tensor, r0 * T2C, [[T2R * T2C, C], [1, (r1 - r0) * T2C]]),
                        bass.AP(t2s.tensor, r0 * T2C, [[T2R * T2C, C], [1, (r1 - r0) * T2C]]),
                    )
                if i < len(t1blks):
                    b0, b1 = t1blks[i]
                    nc.sync.dma_start(
                        bass.AP(t1t.tensor, 128 * b0, [[H * WH, C], [1, (b1 - b0) * 128]]),
                        bass.AP(t1s.tensor, 128 * b0, [[H * WH, C], [1, (b1 - b0) * 128]]),
                    )

            def batch_loop(_iv=None):
                for b in range(NBATCH):
                    gsb = gpool.tile([C, SG], bf, name="gsb")
                    for q in range(IL // 2):  # 2 blocks per PSUM tile/copy
                        ps = pp.tile([128, 1024], mybir.dt.float32, name="ps")
                        SpA = ps.tensor.shape[-1]
                        for jj in range(2):
                            j = 2 * q + jj
                            blk = IL * b + j
                            rb, cb = blk // NBX, blk % NBX
                            lhsT = bass.AP(t1t.tensor, blk * 128, [[H * WH, C], [1, 128]])
                            rhs = bass.AP(
                                t2t.tensor,
                                (BR * rb) * T2C + BC * cb,
                                [[T2R * T2C, C], [T2C, WR], [1, WC]],
                            )
                            nc.tensor.matmul(
                                bass.AP(ps.tensor, 512 * jj, [[SpA, 128], [1, GW]]),
                                lhsT, rhs, start=True, stop=True,
                            )
                        # one 2-block copy amortizes the PSUM access latency;
                        # ACT/DVE alternate pairs (8/8 per batch)
                        eng = nc.scalar.copy if q % 2 == 0 else nc.vector.tensor_copy
                        eng(
                            bass.AP(gsb.tensor, 2 * q, [[SG, 128], [1, 2], [IL, GW]]),
                            bass.AP(ps.tensor, 0, [[SpA, 128], [512, 2], [1, GW]]),
                        )
                    # one dump per 16-partition group: contiguous 5120-elem
                    # runs; group rebase (partitions 16g', band start wcol
                    # 32g' -> col 1024g') lives in the scalar offset.
                    for g in range(NG):
                        # split dumps across the HWDGE (SP) and SWDGE (Pool)
                        # paths so neither queue serializes the batch tail
                        dma = nc.sync.dma_start if g % 2 == 0 else nc.gpsimd.dma_start
                        dma(
                            bass.AP(outp.tensor, (b * NG + g) * GBYTES,
                                    [[BDW, 16], [1, BDW]]),
                            bass.AP(gsb.tensor, 16 * g * SG + (RPG * WC * IL) * g,
                                    [[SG, 16], [1, BDW]]),
                        )

            if reps is None:
                batch_loop()
            else:
                with tc.For_i(0, reps, 1) as iv:
                    batch_loop(iv)

    nc.compile()
    return nc


def _prep_inputs(t1, t2):
    bf16 = ml_dtypes.bfloat16
    in_maps = []
    for k in range(8):
        b, xh = k // 2, k % 2
        xs = xh * WH
        t1c = (t1[b, :, :, xs : xs + WH] * (1.0 / C)).astype(bf16)
        # block-major: t1blk[c, ((rb*NBX + cb)*BR + r)*BC + cx]
        t1blk = np.ascontiguousarray(
            t1c.reshape(C, NBY, BR, NBX, BC).transpose(0, 1, 3, 2, 4)
        ).reshape(C, H * WH)
        t2p = np.zeros((C, T2R, T2C), dtype=bf16)
        lo, hi = max(0, xs - MD), min(W, xs + WH + MD)
        t2p[:, MD : MD + H, lo - (xs - MD) : hi - (xs - MD)] = t2[b, :, :, lo:hi].astype(
            bf16
        )
        in_maps.append({"t1s": t1blk, "t2s": t2p.reshape(C, T2R * T2C)})
    return in_maps


def _unshard(band):
    """band: bf16 [OUTN] for one core -> fp32 [81, H, WH]."""
    it = band.dtype.itemsize
    # axes: (b, g', r'', c, dy, dx, jhi, jlo)
    v = np.lib.stride_tricks.as_strided(
        band,
        shape=(NBATCH, NG, RPG, BC, 2 * MD + 1, 2 * MD + 1, NBY // NBATCH, NBX),
        strides=tuple(
            s * it
            for s in (
                NG * GBYTES,          # batch
                GBYTES,               # group
                BC * BDW + WC * IL,   # r'': 8 partitions + 16 wcols
                BDW + IL,             # c: 1 partition + 1 wcol
                WC * IL,              # dy: 16 wcols
                IL,                   # dx: 1 wcol
                16,                   # jhi: j += 16
                1,                    # jlo: j += 1
            )
        ),
    )
    a = v.astype(np.float32)
    # y = 32b + 16jhi + 2g' + r''; x = 8jlo + c; d = 9dy + dx
    return (
        a.transpose(4, 5, 0, 6, 1, 2, 7, 3)
        .reshape(D, H, WH)
    )


def kernel(t1: np.ndarray, t2: np.ndarray) -> np.ndarray:
    from concourse.bass_utils import run_bass_kernel_spmd

    global _compiled
    if _compiled is None:
        _compiled = _build()
    nc = _compiled

    t1 = np.asarray(t1, dtype=np.float32)
    t2 = np.asarray(t2, dtype=np.float32)
    res = run_bass_kernel_spmd(nc, _prep_inputs(t1, t2), list(range(8)))

    out = np.empty((B, D, H, W), dtype=np.float32)
    for k in range(8):
        b, xh = k // 2, k % 2
        xs = xh * WH
        out[b, :, :, xs : xs + WH] = _unshard(res.results[k]["outp"])
    return out


# revision 11
# speedup vs baseline: 1.2920x; 1.0258x over previous
"""PWC-Net local correlation (MD=4, 81 displacements) on 8 Trainium2 cores.

Problem: t1, t2: [B=4, C=128, H=128, W=256] fp32
  out[b, d, y, x] = mean_c t1[b,c,y,x] * t2pad[b,c,y+dy,x+dx],  d = (dy+4)*9+(dx+4)

Sharding: 8 cores = B(4) x W-half(2); host pre-slices/pads/bf16-casts
(t1 pre-scaled by 1/C so the gram is already the mean).

Per core (128x128 pixels), patch-gram architecture:
  - image tiled into 128 blocks of 16x8 pixels; per block ONE matmul:
    stationary lhsT = t1 block pixels (C x 128, partition p = 8*r + c),
    moving rhs = t2 window (24x16 = 384 cols, via a 3-dim AP into the
    full padded t2 tile). Gram [128 pix, 384] in PSUM.
  - ACT/DVE pair-copies evacuate PSUM fp32 (2 blocks per copy, one
    [128,1024] PSUM tile each, 4-deep rotation) -> shared gsb bf16 tile,
    batch-blocks column-interleaved (dst stride = batch size) so dump
    descriptors are large/contiguous. Subtile deps let both engines and
    chunked input loads share tiles without false serialization.
  - Band dump: per 16-partition group (2 pixel rows), the needed gram
    columns are the contiguous 160-wide (10 window rows x 16) band
    starting at wcol 32*g'. One DMA per (batch, group), alternating
    HWDGE (SP) / SWDGE (Pool) queues; the group rebase lives in the
    scalar offset (per-partition skew is illegal in SBUF-side AP dims;
    scalar offsets may mix partition+column).
  - Batch sizes [16, 48, 48, 16]: small first batch starts the dump
    pipeline early, small last batch shrinks the copy-paced drain tail.
  - The band IS the kernel output (1.98x inflated vs the final 81/pixel);
    host unshard finishes with one as_strided gather per (core, batch)
    (pure indexing - every output value is computed exactly once on
    device; host only selects/arranges, like a transpose).
This replaces the column-gram + DRAM-bounce design: ~14MB DMA/core
instead of ~27MB, ~45 DMA instructions instead of 136+, and 3.2x less
PE + PSUM-evacuation work (384-wide windowed grams vs 1242).
"""

import numpy as np
import ml_dtypes

B, C, H, W = 4, 128, 128, 256
MD = 4
D = (2 * MD + 1) ** 2  # 81
WH = W // 2  # 128 columns per core
BR, BC = 16, 8  # block pixel rows/cols
NBY, NBX = H // BR, WH // BC  # 8 x 16 = 128 blocks
BATCHES = [16, 48, 48, 16]  # blocks per batch (also the interleave factor)
BSTART = [0, 16, 64, 112]
WR, WC = BR + 2 * MD, BC + 2 * MD  # 24 x 16 window
GW = WR * WC  # 384 gram width
T2R = H + 2 * MD  # 136 padded t2 rows
T2C = WH + 2 * MD  # 136 padded t2 cols
RPG = 2  # pixel rows per 16-partition dump group
NG = 128 // (RPG * BC)  # 8 groups
BW = (RPG + 2 * MD) * WC  # 160 band width per partition
# band output: per (batch, group): 16 partitions x BW*ILb elems
BOFF = []  # flat elem offset of each batch's band
_o = 0
for _ib in BATCHES:
    BOFF.append(_o)
    _o += NG * 16 * BW * _ib
OUTN = _o  # 2621440 elems total
_compiled = None


def _build(reps=None):
    """Build the per-core program. reps=None: single pass. reps=R wraps the
    compute in a hardware For loop (benchmarking only)."""
    import concourse.bacc as bacc
    import concourse.bass as bass
    import concourse.mybir as mybir
    import concourse.tile as tile

    bf = mybir.dt.bfloat16
    nc = bacc.Bacc("TRN2", target_bir_lowering=False, debug=False, num_devices=8)
    t1s = nc.dram_tensor("t1s", [C, H * WH], bf, kind="ExternalInput").ap()
    t2s = nc.dram_tensor("t2s", [C, T2R * T2C], bf, kind="ExternalInput").ap()
    outp = nc.dram_tensor("outp", [OUTN], bf, kind="ExternalOutput").ap()

    with tile.TileContext(nc) as tc:
        with (
            tc.tile_pool(name="inputs", bufs=1) as inp,
            tc.tile_pool(name="gpool", bufs=3) as gpool,
            tc.tile_pool(name="psum", bufs=4, space="PSUM") as pp,
        ):
            # one t1 tile + one t2 tile, loaded in fine-grained chunks so
            # batch b's matmuls only wait on the chunks they read (subtile
            # deps). First chunks are small for a fast pipeline start.
            t1t = inp.tile([C, H * WH], bf, name="t1t")
            t2t = inp.tile([C, T2R * T2C], bf, name="t2t")
            t2rows = [(0, 24), (24, 40), (40, 56), (56, 72), (72, 88), (88, 104), (104, 120), (120, 136)]
            t1blks = [(0, 8), (8, 16), (16, 40), (40, 64), (64, 88), (88, 112), (112, 128)]
            for i in range(max(len(t2rows), len(t1blks))):
                if i < len(t2rows):
                    r0, r1 = t2rows[i]
                    nc.sync.dma_start(
                        bass.AP(t2t.tensor, r0 * T2C, [[T2R * T2C, C], [1, (r1 - r0) * T2C]]),
                        bass.AP(t2s.tensor, r0 * T2C, [[T2R * T2C, C], [1, (r1 - r0) * T2C]]),
                    )
                if i < len(t1blks):
                    b0, b1 = t1blks[i]
                    nc.sync.dma_start(
                        bass.AP(t1t.tensor, 128 * b0, [[H * WH, C], [1, (b1 - b0) * 128]]),
                        bass.AP(t1s.tensor, 128 * b0, [[H * WH, C], [1, (b1 - b0) * 128]]),
                    )

            def batch_loop(_iv=None):
                for b, ILb in enumerate(BATCHES):
                    SG = GW * max(BATCHES)  # allocated row pitch (shared slot size)
                    gsb = gpool.tile([C, SG], bf, name="gsb")
                    for q in range(ILb // 2):  # 2 blocks per PSUM tile/copy
                        ps = pp.tile([128, 1024], mybir.dt.float32, name="ps")
                        SpA = ps.tensor.shape[-1]
                        for jj in range(2):
                            j = 2 * q + jj
                            blk = BSTART[b] + j
                            rb, cb = blk // NBX, blk % NBX
                            lhsT = bass.AP(t1t.tensor, blk * 128, [[H * WH, C], [1, 128]])
                            rhs = bass.AP(
                                t2t.tensor,
                                (BR * rb) * T2C + BC * cb,
                                [[T2R * T2C, C], [T2C, WR], [1, WC]],
                            )
                            nc.tensor.matmul(
                                bass.AP(ps.tensor, 512 * jj, [[SpA, 128], [1, GW]]),
                                lhsT, rhs, start=True, stop=True,
                            )
                        # one 2-block copy amortizes the PSUM access latency;
                        # ACT/DVE alternate pairs
                        eng = nc.scalar.copy if q % 2 == 0 else nc.vector.tensor_copy
                        eng(
                            bass.AP(gsb.tensor, 2 * q, [[SG, 128], [1, 2], [ILb, GW]]),
                            bass.AP(ps.tensor, 0, [[SpA, 128], [512, 2], [1, GW]]),
                        )
                    for g in range(NG):
                        # split dumps across the HWDGE (SP) and SWDGE (Pool)
                        # paths so neither queue serializes the batch tail
                        dma = nc.sync.dma_start if g % 2 == 0 else nc.gpsimd.dma_start
                        dma(
                            bass.AP(outp.tensor, BOFF[b] + g * 16 * BW * ILb,
                                    [[BW * ILb, 16], [1, BW * ILb]]),
                            bass.AP(gsb.tensor, 16 * g * SG + (RPG * WC * ILb) * g,
                                    [[SG, 16], [1, BW * ILb]]),
                        )

            if reps is None:
                batch_loop()
            else:
                with tc.For_i(0, reps, 1) as iv:
                    batch_loop(iv)

    nc.compile()
    return nc


def _prep_inputs(t1, t2):
    bf16 = ml_dtypes.bfloat16
    in_maps = []
    for k in range(8):
        b, xh = k // 2, k % 2
        xs = xh * WH
        t1c = (t1[b, :, :, xs : xs + WH] * (1.0 / C)).astype(bf16)
        # block-major: t1blk[c, ((rb*NBX + cb)*BR + r)*BC + cx]
        t1blk = np.ascontiguousarray(
            t1c.reshape(C, NBY, BR, NBX, BC).transpose(0, 1, 3, 2, 4)
        ).reshape(C, H * WH)
        t2p = np.zeros((C, T2R, T2C), dtype=bf16)
        lo, hi = max(0, xs - MD), min(W, xs + WH + MD)
        t2p[:, MD : MD + H, lo - (xs - MD) : hi - (xs - MD)] = t2[b, :, :, lo:hi].astype(
            bf16
        )
        in_maps.append({"t1s": t1blk, "t2s": t2p.reshape(C, T2R * T2C)})
    return in_maps


def _unshard(band):
    """band: bf16 [OUTN] for one core -> fp32 [81, H, WH]."""
    it = band.dtype.itemsize
    out = np.empty((D, H, WH), dtype=np.float32)
    for b, ILb in enumerate(BATCHES):
        BDW = BW * ILb
        # axes: (g', r'', c, dy, dx, jhi, jlo); j = 16*jhi + jlo
        v = np.lib.stride_tricks.as_strided(
            band[BOFF[b] :],
            shape=(NG, RPG, BC, 2 * MD + 1, 2 * MD + 1, ILb // 16, NBX),
            strides=tuple(
                s * it
                for s in (
                    16 * BDW,            # group
                    BC * BDW + WC * ILb, # r'': 8 partitions + 16 wcols
                    BDW + ILb,           # c: 1 partition + 1 wcol
                    WC * ILb,            # dy: 16 wcols
                    ILb,                 # dx: 1 wcol
                    16,                  # jhi: j += 16
                    1,                   # jlo: j += 1
                )
            ),
        )
        a = v.astype(np.float32)
        # y = 16*(BSTART[b]//16 + jhi) + 2g' + r''; x = 8jlo + c; d = 9dy+dx
        nrb = ILb // 16
        y0 = BR * (BSTART[b] // 16)
        out[:, y0 : y0 + BR * nrb, :] = a.transpose(3, 4, 5, 0, 1, 6, 2).reshape(
            D, BR * nrb, WH
        )
    return out


def kernel(t1: np.ndarray, t2: np.ndarray) -> np.ndarray:
    from concourse.bass_utils import run_bass_kernel_spmd

    global _compiled
    if _compiled is None:
        _compiled = _build()
    nc = _compiled

    t1 = np.asarray(t1, dtype=np.float32)
    t2 = np.asarray(t2, dtype=np.float32)
    res = run_bass_kernel_spmd(nc, _prep_inputs(t1, t2), list(range(8)))

    out = np.empty((B, D, H, W), dtype=np.float32)
    for k in range(8):
        b, xh = k // 2, k % 2
        xs = xh * WH
        out[b, :, :, xs : xs + WH] = _unshard(res.results[k]["outp"])
    return out


# revision 16
# speedup vs baseline: 1.3228x; 1.0238x over previous
"""PWC-Net local correlation (MD=4, 81 displacements) on 8 Trainium2 cores.

Problem: t1, t2: [B=4, C=128, H=128, W=256] fp32
  out[b, d, y, x] = mean_c t1[b,c,y,x] * t2pad[b,c,y+dy,x+dx],  d = (dy+4)*9+(dx+4)

Sharding: 8 cores = B(4) x W-half(2); host pre-slices/pads/bf16-casts
(t1 pre-scaled by 1/C so the gram is already the mean).

Per core (128x128 pixels), patch-gram architecture:
  - image tiled into 128 blocks of 16x8 pixels; per block ONE matmul:
    stationary lhsT = t1 block pixels (C x 128, partition p = 8*r + c),
    moving rhs = t2 window (24x16 = 384 cols, via a 3-dim AP into the
    full padded t2 tile). Gram [128 pix, 384] in PSUM.
  - ACT/DVE pair-copies evacuate PSUM fp32 (2 blocks per copy, one
    [128,1024] PSUM tile each, 4-deep rotation) -> shared gsb bf16 tile,
    batch-blocks column-interleaved (dst stride = batch size) so dump
    descriptors are large/contiguous. Subtile deps let both engines and
    chunked input loads share tiles without false serialization.
  - Band dump: per 16-partition group (2 pixel rows), the needed gram
    columns are the contiguous 160-wide (10 window rows x 16) band
    starting at wcol 32*g'. One DMA per (batch, group), alternating
    HWDGE (SP) / SWDGE (Pool) queues; the group rebase lives in the
    scalar offset (per-partition skew is illegal in SBUF-side AP dims;
    scalar offsets may mix partition+column).
  - Batch sizes [16, 48, 48, 16]: small first batch starts the dump
    pipeline early, small last batch shrinks the copy-paced drain tail.
  - The band IS the kernel output (1.98x inflated vs the final 81/pixel);
    host unshard finishes with one as_strided gather per (core, batch)
    (pure indexing - every output value is computed exactly once on
    device; host only selects/arranges, like a transpose).
This replaces the column-gram + DRAM-bounce design: ~14MB DMA/core
instead of ~27MB, ~45 DMA instructions instead of 136+, and 3.2x less
PE + PSUM-evacuation work (384-wide windowed grams vs 1242).
"""

import numpy as np
import ml_dtypes

B, C, H, W = 4, 128, 128, 256
MD = 4
D = (2 * MD + 1) ** 2  # 81
WH = W // 2  # 128 columns per core
BR, BC = 16, 8  # block pixel rows/cols
NBY, NBX = H // BR, WH // BC  # 8 x 16 = 128 blocks
BATCHES = [16, 48, 48, 16]  # blocks per batch (also the interleave factor)
BSTART = [0, 16, 64, 112]
WR, WC = BR + 2 * MD, BC + 2 * MD  # 24 x 16 window
GW = WR * WC  # 384 gram width
T2R = H + 2 * MD  # 136 padded t2 rows
T2C = WH + 2 * MD  # 136 padded t2 cols
RPG = 2  # pixel rows per dump group (last batch uses 2*RPG)
BRPG = [RPG, RPG, RPG, 2 * RPG]  # per-batch group height
WC9 = 2 * MD + 1
# band width per partition = (rpg + 2*MD) * WC; output offsets per batch
BOFF = []
_o = 0
for _b, _ib in enumerate(BATCHES):
    BOFF.append(_o)
    _o += 128 * (BRPG[_b] + 2 * MD) * WC * _ib
OUTN = _o
_compiled = None


def _build(reps=None):
    """Build the per-core program. reps=None: single pass. reps=R wraps the
    compute in a hardware For loop (benchmarking only)."""
    import concourse.bacc as bacc
    import concourse.bass as bass
    import concourse.mybir as mybir
    import concourse.tile as tile

    bf = mybir.dt.bfloat16
    nc = bacc.Bacc("TRN2", target_bir_lowering=False, debug=False, num_devices=8)
    t1s = nc.dram_tensor("t1s", [C, H * WH], bf, kind="ExternalInput").ap()
    t2s = nc.dram_tensor("t2s", [C, T2R * T2C], bf, kind="ExternalInput").ap()
    outp = nc.dram_tensor("outp", [OUTN], bf, kind="ExternalOutput").ap()

    with tile.TileContext(nc) as tc:
        with (
            tc.tile_pool(name="inputs", bufs=1) as inp,
            tc.tile_pool(name="gpool", bufs=3) as gpool,
            tc.tile_pool(name="psum", bufs=4, space="PSUM") as pp,
        ):
            # one t1 tile + one t2 tile, loaded in fine-grained chunks so
            # batch b's matmuls only wait on the chunks they read (subtile
            # deps). First chunks are small for a fast pipeline start.
            t1t = inp.tile([C, H * WH], bf, name="t1t")
            t2t = inp.tile([C, T2R * T2C], bf, name="t2t")
            # y-pad rows are zero: memset on the idle Pool engine instead of
            # DMAing 278KB of zeros
            nc.gpsimd.memset(bass.AP(t2t.tensor, 0, [[T2R * T2C, C], [1, 4 * T2C]]), 0.0)
            nc.gpsimd.memset(
                bass.AP(t2t.tensor, 132 * T2C, [[T2R * T2C, C], [1, 4 * T2C]]), 0.0
            )
            t2rows = [(4, 24), (24, 40), (40, 56), (56, 72), (72, 88), (88, 104), (104, 120), (120, 132)]
            t1blks = [(0, 8), (8, 16), (16, 40), (40, 64), (64, 88), (88, 112), (112, 128)]
            for i in range(max(len(t2rows), len(t1blks))):
                if i < len(t2rows):
                    r0, r1 = t2rows[i]
                    nc.sync.dma_start(
                        bass.AP(t2t.tensor, r0 * T2C, [[T2R * T2C, C], [1, (r1 - r0) * T2C]]),
                        bass.AP(t2s.tensor, r0 * T2C, [[T2R * T2C, C], [1, (r1 - r0) * T2C]]),
                    )
                if i < len(t1blks):
                    b0, b1 = t1blks[i]
                    nc.sync.dma_start(
                        bass.AP(t1t.tensor, 128 * b0, [[H * WH, C], [1, (b1 - b0) * 128]]),
                        bass.AP(t1s.tensor, 128 * b0, [[H * WH, C], [1, (b1 - b0) * 128]]),
                    )

            def batch_loop(_iv=None):
                for b, ILb in enumerate(BATCHES):
                    SG = GW * max(BATCHES)  # allocated row pitch (shared slot size)
                    gsb = gpool.tile([C, SG], bf, name="gsb")
                    for q in range(ILb // 2):  # 2 blocks per PSUM tile/copy
                        ps = pp.tile([128, 1024], mybir.dt.float32, name="ps")
                        SpA = ps.tensor.shape[-1]
                        for jj in range(2):
                            j = 2 * q + jj
                            blk = BSTART[b] + j
                            rb, cb = blk // NBX, blk % NBX
                            lhsT = bass.AP(t1t.tensor, blk * 128, [[H * WH, C], [1, 128]])
                            rhs = bass.AP(
                                t2t.tensor,
                                (BR * rb) * T2C + BC * cb,
                                [[T2R * T2C, C], [T2C, WR], [1, WC]],
                            )
                            nc.tensor.matmul(
                                bass.AP(ps.tensor, 512 * jj, [[SpA, 128], [1, GW]]),
                                lhsT, rhs, start=True, stop=True,
                            )
                        # one 2-block copy amortizes the PSUM access latency;
                        # ACT/DVE alternate pairs
                        eng = nc.scalar.copy if q % 2 == 0 else nc.vector.tensor_copy
                        eng(
                            bass.AP(gsb.tensor, 2 * q, [[SG, 128], [1, 2], [ILb, GW]]),
                            bass.AP(ps.tensor, 0, [[SpA, 128], [512, 2], [1, GW]]),
                        )
                    # last batch: 32-partition groups (4 bigger dumps) so the
                    # drain tail isn't serialized on 8 small DMAs
                    rpg = BRPG[b]
                    ng = 128 // (rpg * BC)
                    bw = (rpg + 2 * MD) * WC
                    for g in range(ng):
                        # split dumps across the HWDGE (SP) and SWDGE (Pool)
                        # paths so neither queue serializes the batch tail
                        dma = nc.sync.dma_start if g % 2 == 0 else nc.gpsimd.dma_start
                        dma(
                            bass.AP(outp.tensor, BOFF[b] + g * rpg * BC * bw * ILb,
                                    [[bw * ILb, rpg * BC], [1, bw * ILb]]),
                            bass.AP(gsb.tensor, rpg * BC * g * SG + (rpg * WC * ILb) * g,
                                    [[SG, rpg * BC], [1, bw * ILb]]),
                        )

            if reps is None:
                batch_loop()
            else:
                with tc.For_i(0, reps, 1) as iv:
                    batch_loop(iv)

    nc.compile()
    return nc


def _prep_inputs(t1, t2):
    bf16 = ml_dtypes.bfloat16
    in_maps = []
    for k in range(8):
        b, xh = k // 2, k % 2
        xs = xh * WH
        t1c = (t1[b, :, :, xs : xs + WH] * (1.0 / C)).astype(bf16)
        # block-major: t1blk[c, ((rb*NBX + cb)*BR + r)*BC + cx]
        t1blk = np.ascontiguousarray(
            t1c.reshape(C, NBY, BR, NBX, BC).transpose(0, 1, 3, 2, 4)
        ).reshape(C, H * WH)
        t2p = np.zeros((C, T2R, T2C), dtype=bf16)
        lo, hi = max(0, xs - MD), min(W, xs + WH + MD)
        t2p[:, MD : MD + H, lo - (xs - MD) : hi - (xs - MD)] = t2[b, :, :, lo:hi].astype(
            bf16
        )
        in_maps.append({"t1s": t1blk, "t2s": t2p.reshape(C, T2R * T2C)})
    return in_maps


def _unshard(band):
    """band: bf16 [OUTN] for one core -> fp32 [81, H, WH]."""
    it = band.dtype.itemsize
    out = np.empty((D, H, WH), dtype=np.float32)
    for b, ILb in enumerate(BATCHES):
        rpg = BRPG[b]
        ng = 128 // (rpg * BC)
        bw = (rpg + 2 * MD) * WC
        BDW = bw * ILb
        # axes: (g', r'', c, dy, dx, jhi, jlo); j = 16*jhi + jlo
        v = np.lib.stride_tricks.as_strided(
            band[BOFF[b] :],
            shape=(ng, rpg, BC, WC9, WC9, ILb // 16, NBX),
            strides=tuple(
                s * it
                for s in (
                    rpg * BC * BDW,      # group: rpg*8 partitions
                    BC * BDW + WC * ILb, # r'': 8 partitions + 16 wcols
                    BDW + ILb,           # c: 1 partition + 1 wcol
                    WC * ILb,            # dy: 16 wcols
                    ILb,                 # dx: 1 wcol
                    16,                  # jhi: j += 16
                    1,                   # jlo: j += 1
                )
            ),
        )
        a = v.astype(np.float32)
        # y = 16*(BSTART[b]//16 + jhi) + rpg*g' + r''; x = 8jlo + c
        nrb = ILb // 16
        y0 = BR * (BSTART[b] // 16)
        out[:, y0 : y0 + BR * nrb, :] = a.transpose(3, 4, 5, 0, 1, 6, 2).reshape(
            D, BR * nrb, WH
        )
    return out


def kernel(t1: np.ndarray, t2: np.ndarray) -> np.ndarray:
    from concourse.bass_utils import run_bass_kernel_spmd

    global _compiled
    if _compiled is None:
        _compiled = _build()
    nc = _compiled

    t1 = np.asarray(t1, dtype=np.float32)
    t2 = np.asarray(t2, dtype=np.float32)
    res = run_bass_kernel_spmd(nc, _prep_inputs(t1, t2), list(range(8)))

    out = np.empty((B, D, H, W), dtype=np.float32)
    for k in range(8):
        b, xh = k // 2, k % 2
        xs = xh * WH
        out[b, :, :, xs : xs + WH] = _unshard(res.results[k]["outp"])
    return out
